# revision 28
# baseline (speedup 1.0000x reference)
"""Trainium2 Bass kernel for DiagTrainableLDAHead (retrieval_knn).

out[n,c] = log_prior[c] - 0.5*(m2[n,c] + log_det)
m2[n,c]  = sum_d (z[n,d]-mu[c,d])^2 * inv_var[d]
=> out[n,c] = cross[n,c] + rb[n] + cb[c]
   cross = (z*inv_var) @ mu^T      (fp8e4 DoubleRow GEMM, ~155 TF/s)
   rb[n] = -0.5 * sum_d z^2 inv_var          (fp16 PE reduce)
   cb[c] = log_prior[c] - 0.5*(mu_sq[c]+log_det)  (fp8 DR PE reduce)

rb/cb ride a third DoubleRow matmul per tile on augmented fp8 operands
(rb/8 split hi+mid with partner 8 at partitions 0; cb single fp8 at
partition 32/...), so PSUM holds the finished output and eviction is a
bare fp32->fp16 copy.

Measured-cost driven structure (see NTFF analyses):
 - DR matmul N=512: 216ns spacing warm; PE stream 96 MMs ~21us.
 - psum eviction [128,1024] ~1.0-1.3us/op; total evict ~19us MUST be
   split across scalar+vector(+gpsimd probe) -> double-bank psum tiles.
 - single-partition [1,512] ops ~0.5-0.8us each: chains are minimized
   (cb = 2 ops/cj, rb = 4 ops/ch) and write fp8 rows directly.
 - mu/z squares ~0.6ns/elem/partition on scalar or DVE; split evenly.
 - act tables (EXP/LN) preloaded via dummy ops before DMA arrives.
 - first DMA byte ~8.7us always; 12 warmup matmuls keep HAM at 2.4GHz.
 - all outputs on the sync ring (dma_start sem-waits head-of-line
   block their queue; sync is otherwise idle mid-kernel).

Sharding: data-parallel over N across 8 cores; mu/log_cov/prior
replicated; forward-only, no collectives.  Host prep is layout/dtype
only; all input-dependent arithmetic is on-device.
"""
import sys

sys.path.insert(0, "/opt/trn_rl_repo")

import ml_dtypes
import numpy as np

import concourse.bacc as bacc
import concourse.tile as tile
from concourse import mybir
from concourse.bass_utils import run_bass_kernel_spmd

F32 = mybir.dt.float32
F16 = mybir.dt.float16
BF16 = mybir.dt.bfloat16
F8 = mybir.dt.float8e4
AF = mybir.ActivationFunctionType
ALU = mybir.AluOpType
DR = mybir.MatmulPerfMode.DoubleRow

N, C, D = 8192, 2048, 512
NCORES = 8
NSH = N // NCORES
P = 128
NCH = 2
KT = 4
KT2 = 2
CJ = 4
F = 512
NT = NSH // P
PJ = C // P
CH2 = C // 2

_CACHE = {}


def _build():
    nc = bacc.Bacc("TRN2", target_bir_lowering=False, debug=False,
                   enable_asserts=False, num_devices=NCORES)

    zB = nc.dram_tensor("zB", [P, NCH, KT, F], BF16, kind="ExternalInput").ap()
    m8 = nc.dram_tensor("m8", [P, KT2, 2, C], F8, kind="ExternalInput").ap()
    zaugT = nc.dram_tensor("zaugT", [P, 2, NSH], F8, kind="ExternalInput").ap()
    baugT = nc.dram_tensor("baugT", [P, 2, C], F8, kind="ExternalInput").ap()
    lc = nc.dram_tensor("lc", [1, D], F32, kind="ExternalInput").ap()
    prior = nc.dram_tensor("prior", [1, C], F32, kind="ExternalInput").ap()
    pr2d = nc.dram_tensor("pr2d", [P, PJ], F32, kind="ExternalInput").ap()
    outW = nc.dram_tensor("outW", [P, NT, C], F16, kind="ExternalOutput").ap()

    with tile.TileContext(nc) as tc:
        with (
            tc.tile_pool(name="const", bufs=1) as const,
            tc.tile_pool(name="stage", bufs=4) as stage,
            tc.tile_pool(name="psS", bufs=2, space="PSUM") as psS,
            tc.tile_pool(name="psM", bufs=3, space="PSUM") as psM,
        ):
            # ---- input DMAs -------------------------------------------
            lc_f = const.tile([1, D], F32)
            pr2 = const.tile([P, PJ], F32)
            pr = const.tile([1, C], F32)
            zF = const.tile([P, NCH, KT, F], BF16)
            m8s = const.tile([P, KT2, 2, C], F8)
            zaug = const.tile([P, 2, NSH], F8)
            baug = const.tile([P, 2, C], F8)
            nc.sync.dma_start(out=lc_f[:], in_=lc[:, :])
            nc.sync.dma_start(out=pr2[:], in_=pr2d[:, :])
            nc.sync.dma_start(out=pr[:], in_=prior[:, :])
            nc.sync.dma_start(out=zF[:, 0, 0:2, :], in_=zB[:, 0, 0:2, :])
            nc.sync.dma_start(out=m8s[:, 0, :, 0:CH2], in_=m8[:, 0, :, 0:CH2])
            nc.sync.dma_start(out=zF[:, 0, 2:4, :], in_=zB[:, 0, 2:4, :])
            nc.sync.dma_start(out=m8s[:, 0, :, CH2:C], in_=m8[:, 0, :, CH2:C])
            # (ordering note: zB0/m80 interleaved so z8-kt01 + mains-cj01
            #  can start ~11.5us; mq halves pace the first bias ~15.5us)
            nc.scalar.dma_start(out=m8s[:, 1, :, 0:CH2], in_=m8[:, 1, :, 0:CH2])
            nc.scalar.dma_start(out=m8s[:, 1, :, CH2:C], in_=m8[:, 1, :, CH2:C])
            nc.scalar.dma_start(out=zF[:, 1, 0:2, :], in_=zB[:, 1, 0:2, :])
            nc.scalar.dma_start(out=zF[:, 1, 2:4, :], in_=zB[:, 1, 2:4, :])
            nc.gpsimd.dma_start(out=zaug[:], in_=zaugT[:, :])
            nc.gpsimd.dma_start(out=baug[:], in_=baugT[:, :])

            # ---- tiny consts + act-table warmers ----------------------
            wz = const.tile([P, F], BF16)
            nc.vector.memset(wz[:], 0.0)
            id1 = const.tile([1, 1], F32)
            nc.vector.memset(id1[:], 1.0)
            ones32 = const.tile([P, 1], F32)
            nc.vector.memset(ones32[:], 1.0)
            tw = const.tile([1, 1], F32)
            nc.scalar.activation(tw[:], id1[:], AF.Exp)     # load EXP table
            nc.scalar.activation(tw[:], id1[:], AF.Ln)      # load LN table

            # ---- PE warmup --------------------------------------------
            def warm(k):
                for _ in range(k):
                    pw = psS.tile([8, F], F32, tag="setup", name="pw")
                    nc.tensor.matmul(pw[:], lhsT=wz[:, 0:8], rhs=wz[:],
                                     start=True, stop=True)

            warm(6)
            plc = psS.tile([P, KT], F32, tag="setup")
            for kt in range(KT):
                nc.tensor.transpose(plc[:, kt:kt + 1],
                                    lc_f[:, kt * P:(kt + 1) * P], id1[:])
            warm(2)

            # ---- scalar: iv chain; DVE: ldp/sexp ----------------------
            lc_p = const.tile([P, KT], F32)
            nc.scalar.copy(lc_p[:], plc[:])
            iv = const.tile([P, KT], F32)
            nc.scalar.activation(iv[:], lc_p[:], AF.Exp, scale=-1.0)
            iv16 = const.tile([P, KT], F16)
            nc.scalar.copy(iv16[:], iv[:])
            iv8 = const.tile([P, KT2, 2, 16], F8)
            for kt in range(KT):
                nc.scalar.copy(iv8[:, kt // 2, kt % 2, 0:1], iv[:, kt:kt + 1])
            pex = const.tile([P, PJ], F32)
            nc.scalar.activation(pex[:], pr2[:], AF.Exp)

            ldp = const.tile([P, 1], F32)
            nc.vector.tensor_reduce(out=ldp[:], in_=lc_p[:],
                                    axis=mybir.AxisListType.X, op=ALU.add)

            # ---- z/mu prep: zq+z8 all on DVE; mq split ----------------
            zq = const.tile([P, NCH, KT, F], F16)
            z8 = const.tile([P, NCH, KT2, 2, F], F8)
            mq = const.tile([P, KT2, 2, C], F8)

            def prep_z(ch):
                # z8 first (gates the GEMM), zq after (gates only rb)
                for kt in range(KT):
                    nc.vector.tensor_scalar_mul(
                        z8[:, ch, kt // 2, kt % 2, :], zF[:, ch, kt, :],
                        iv[:, kt:kt + 1])
                nc.vector.tensor_tensor(zq[:, ch, 0:2, :], zF[:, ch, 0:2, :],
                                        zF[:, ch, 0:2, :], ALU.mult)
                nc.vector.tensor_tensor(zq[:, ch, 2:4, :], zF[:, ch, 2:4, :],
                                        zF[:, ch, 2:4, :], ALU.mult)

            prep_z(0)
            # mu squares: k1 on scalar (lands first), k0 on DVE
            nc.scalar.activation(mq[:, 1, :, 0:CH2], m8s[:, 1, :, 0:CH2],
                                 AF.Square)
            nc.scalar.activation(mq[:, 1, :, CH2:C], m8s[:, 1, :, CH2:C],
                                 AF.Square)
            nc.vector.tensor_tensor(mq[:, 0, :, 0:CH2], m8s[:, 0, :, 0:CH2],
                                    m8s[:, 0, :, 0:CH2], ALU.mult)
            nc.vector.tensor_tensor(mq[:, 0, :, CH2:C], m8s[:, 0, :, CH2:C],
                                    m8s[:, 0, :, CH2:C], ALU.mult)

            # ---- PE: pld/pse, z_sq ch0, mu_sq k1 ----------------------
            pld = psS.tile([1, 1], F32, tag="setup")
            nc.tensor.matmul(pld[:], lhsT=ldp[:], rhs=ones32[:],
                             start=True, stop=True)
            sexp = const.tile([P, 1], F32)
            nc.vector.tensor_reduce(out=sexp[:], in_=pex[:],
                                    axis=mybir.AxisListType.X, op=ALU.add)
            pse = psS.tile([1, 1], F32, tag="setup")
            nc.tensor.matmul(pse[:], lhsT=sexp[:], rhs=ones32[:],
                             start=True, stop=True)
            pms = [psS.tile([1, F], F32, tag="setup", name=f"pm{cj}")
                   for cj in range(CJ)]

            def pm_k1(cjs):
                for cj in cjs:
                    nc.tensor.matmul(pms[cj][:], lhsT=iv8[:, 1, :, 0:1],
                                     rhs=mq[:, 1, :, cj * F:(cj + 1) * F],
                                     start=True, stop=False, perf_mode=DR)

            def pm_k0(cjs):
                for cj in cjs:
                    nc.tensor.matmul(pms[cj][:], lhsT=iv8[:, 0, :, 0:1],
                                     rhs=mq[:, 0, :, cj * F:(cj + 1) * F],
                                     start=False, stop=True, perf_mode=DR)


            # ---- nbb --------------------------------------------------
            lse = const.tile([1, 1], F32)
            nc.scalar.activation(lse[:], pse[:], AF.Ln)
            nldh = const.tile([1, 1], F32)
            nc.scalar.mul(nldh[:], pld[:], -0.5)
            nbb = const.tile([1, 1], F32)
            nc.scalar.activation(nbb[:], lse[:], AF.Identity, bias=nldh[:],
                                 scale=-1.0)

            # ---- rb chain: scalar writes hi8 direct, DVE does residual
            m32 = const.tile([1, NSH], F32)

            def rb_chain(ch, pz):
                s = slice(ch * F, (ch + 1) * F)
                nc.scalar.activation(zaug[0:1, 0, s], pz[:], AF.Copy,
                                     scale=-0.0625)
                nc.vector.tensor_scalar_mul(m32[:, s], pz[:], -0.0625)
                nc.vector.tensor_tensor(m32[:, s], m32[:, s], zaug[0:1, 0, s],
                                        ALU.subtract)
                nc.scalar.copy(zaug[0:1, 1, s], m32[:, s])

            # ---- cb chain: 2 ops per cj (single-fp8 cb) ---------------
            cbt = const.tile([1, C], F32)

            def cb_chain(cjs):
                for cj in cjs:
                    s = slice(cj * F, (cj + 1) * F)
                    nc.scalar.activation(cbt[:, s], pms[cj][:], AF.Identity,
                                         bias=nbb[:], scale=-0.5)
                    nc.vector.tensor_tensor(baug[32:33, 1, s], cbt[:, s],
                                            pr[:, s], ALU.add)

            # ---- main tiles: double-bank psums, pipelined bias --------
            open_tiles = []
            ev_rr = [0]

            def emit_k(ni):
                ch, t = ni // KT, ni % KT
                ns = slice(t * P, (t + 1) * P)
                plo = psM.tile([P, 2 * F], F32, name="plo", tag="mm")
                phi = psM.tile([P, 2 * F], F32, name="phi", tag="mm")
                halves = [plo[:, 0:F], plo[:, F:2 * F],
                          phi[:, 0:F], phi[:, F:2 * F]]
                for k2 in range(KT2):
                    for cj in range(CJ):
                        nc.tensor.matmul(
                            halves[cj],
                            lhsT=z8[:, ch, k2, :, ns],
                            rhs=m8s[:, k2, :, cj * F:(cj + 1) * F],
                            start=(k2 == 0), stop=False, perf_mode=DR)
                open_tiles.append((ni, plo, phi, halves))

            def emit_bias_evict():
                ni, plo, phi, halves = open_tiles.pop(0)
                for cj in range(CJ):
                    nc.tensor.matmul(
                        halves[cj],
                        lhsT=zaug[:, :, ni * P:(ni + 1) * P],
                        rhs=baug[:, :, cj * F:(cj + 1) * F],
                        start=False, stop=True, perf_mode=DR)
                st = stage.tile([P, C], F16, tag="st", name="st")
                # evict halves alternate scalar/vector (GPSIMD can't
                # touch PSUM — verifier-enforced)
                for half, src in ((0, plo), (1, phi)):
                    r = ev_rr[0] % 2
                    ev_rr[0] += 1
                    dst = st[:, half * 2 * F:(half + 1) * 2 * F]
                    if r == 0:
                        nc.scalar.copy(dst, src[:])
                    else:
                        nc.vector.tensor_copy(out=dst, in_=src[:])
                nc.sync.dma_start(out=outW[:, ni, :], in_=st[:])

            emit_k(0)
            emit_k(1)
            pz0 = psS.tile([1, F], F32, tag="setup", name="pz0")
            for kt in range(KT):
                nc.tensor.matmul(pz0[:], lhsT=iv16[:, kt:kt + 1],
                                 rhs=zq[:, 0, kt, :],
                                 start=(kt == 0), stop=(kt == KT - 1))
            pm_k1((0, 1))
            pm_k0((0, 1))
            cb_chain((0, 1))
            rb_chain(0, pz0)
            prep_z(1)
            pm_k1((2, 3))
            pm_k0((2, 3))
            cb_chain((2, 3))
            emit_bias_evict()           # ni0
            emit_k(2)
            emit_bias_evict()           # ni1
            emit_k(3)
            pz1 = psS.tile([1, F], F32, tag="setup", name="pz1")
            for kt in range(KT):
                nc.tensor.matmul(pz1[:], lhsT=iv16[:, kt:kt + 1],
                                 rhs=zq[:, 1, kt, :],
                                 start=(kt == 0), stop=(kt == KT - 1))
            rb_chain(1, pz1)
            emit_bias_evict()           # ni2
            for ni in range(KT, NT):
                emit_k(ni)
                emit_bias_evict()
            emit_bias_evict()           # ni7

    nc.compile()
    return nc


def _get_nc():
    if "nc" not in _CACHE:
        _CACHE["nc"] = _build()
    return _CACHE["nc"]


def _aug_templates():
    """Constant fp8 templates: zeros + slot constants.
    slot (Ki,i): (0,0) rb_hi*8  (0,1) rb_mid*8  (32,1) 1*cb"""
    e = ml_dtypes.float8_e4m3
    za = np.zeros((P, 2, NSH), dtype=e)
    za[32, 1, :] = e(1.0)
    ba = np.zeros((P, 2, C), dtype=e)
    ba[0, :, :] = e(8.0)
    return za, ba


def _in_maps(z, mu, log_cov_diag, prior_logits):
    z = np.ascontiguousarray(np.asarray(z, dtype=np.float32))
    mu = np.asarray(mu, dtype=np.float32)
    lcv = np.asarray(log_cov_diag, dtype=np.float32).reshape(1, D)
    pl = np.asarray(prior_logits, dtype=np.float32).reshape(1, C)
    pr2 = np.ascontiguousarray(pl.reshape(PJ, P).T)
    m8c = np.ascontiguousarray(
        mu.reshape(C, KT2, 2, P).transpose(3, 1, 2, 0)
    ).astype(ml_dtypes.float8_e4m3)
    za, ba = _aug_templates()
    maps = []
    for cc in range(NCORES):
        zc = z[cc * NSH:(cc + 1) * NSH]
        zBc = np.ascontiguousarray(
            zc.reshape(NCH, F, KT, P).transpose(3, 0, 2, 1)
        ).astype(ml_dtypes.bfloat16)
        maps.append({"zB": zBc, "m8": m8c, "zaugT": za, "baugT": ba,
                     "lc": lcv, "prior": pl, "pr2d": pr2})
    return maps


def _run(z, mu, log_cov_diag, prior_logits, trace=False, **kw):
    nc = _get_nc()
    maps = _in_maps(z, mu, log_cov_diag, prior_logits)
    res = run_bass_kernel_spmd(nc, maps, list(range(NCORES)), trace=trace, **kw)
    parts = []
    for cc in range(NCORES):
        o = res.results[cc]["outW"]
        parts.append(np.asarray(o).transpose(1, 0, 2).reshape(NSH, C))
    full = np.concatenate(parts, axis=0).astype(np.float32)
    return full, res


def kernel(z, mu, log_cov_diag, prior_logits):
    full, _ = _run(z, mu, log_cov_diag, prior_logits)
    return full
